# revision 32
# baseline (speedup 1.0000x reference)
"""Trainium2 Bass kernel for a Bahdanau-attention GRU decoder.

Reference computation (T=512, B=128, I=H=512, O=12, L=max_labels=16):
    s0 = tanh(x[0] @ ws);  out0 = s0 @ fc_w + fc_b
    U  = einsum('tbi,ih->tbh', x, ua)            # precomputed once
    per step:
        e  = einsum('tbh,h->tb', tanh(s @ wa + U), va)
        a  = softmax(e, axis=t)
        c  = einsum('tb,tbi->bi', a, x)
        r  = sigmoid(out @ wr + s @ ur + c @ cr)
        z  = sigmoid(out @ wz + s @ uz + c @ cz)
        sh = tanh(out @ w0 + (r*s) @ u0 + c @ c0)
        s  = (1-z)*s + z*sh;  out = s @ fc_w + fc_b
    returns [B, L, O]

Sharding: data-parallel over batch B across 8 cores (BL=16 per core), all
weights replicated; no collectives.  Per core, x (fp16, [t%128, b, t//128, i])
and U (fp16, [h%128, h//128, t, b]) are SBUF-resident so the recurrence never
touches HBM.

Step-loop engine split (ACT-bound; tanh of [T,BL,H] is the floor):
  DVE : V = U + broadcast(sWa) slabs (fp16 2x), softmax scalars, gate combines
  ACT : tanh(V) on [128, 256*16] slabs; exp; gate tanh (sigmoid via tanh)
  Pool: small PSUM->SBUF copies + gate adds (keeps DVE under the ACT floor)
  PE  : everything in transposed [h, b] space so matmuls are cheap:
        e-dot via lhsT=va chunk (m=1, N=256); context via lhsT=x chunks
        (Ldweights are engine-free, N=1); gate matmuls lhsT=weight chunks
        rhs=state columns (N=16); s_newT produced directly (no transposes).
"""

import numpy as np
from contextlib import ExitStack

import concourse.bass as bass
import concourse.mybir as mybir
import concourse.tile as tile
from concourse import bacc
from concourse.bass_utils import run_bass_kernel_spmd
from concourse.masks import make_identity

F32 = mybir.dt.float32
F16 = mybir.dt.float16
AF = mybir.ActivationFunctionType
ALU = mybir.AluOpType
AX = mybir.AxisListType

T, B, I, H, O = 512, 128, 512, 512, 12
P = 128
NCORES = 8
BL = B // NCORES        # 16 batches per core
HC = H // P             # 4 h-chunks
IC = I // P             # 4 i-chunks
TC = T // P             # 4 t-chunks
NTH = 2                 # t-halves for the attention slabs
THL = T // NTH          # 256

WNAMES = ["w0", "wz", "wr", "ws", "wa", "ua", "va", "u0", "uz", "ur",
          "c0", "cz", "cr", "fc_w", "fc_b"]


def _load_weight_pkh(nc, pool, wname, ap, kc, cast_pool, dtype=F16):
    """DRAM [K, H] fp32 -> SBUF [P, kc, H] in `dtype` (cast via DVE copy)."""
    w16 = pool.tile([P, kc, H], dtype, name=f"{wname}_sb")
    ap3 = ap.rearrange("(c p) h -> p c h", p=P)
    for c in range(kc):
        tmp = cast_pool.tile([P, H], F32, tag="wload",
                             name=f"{wname}_f32tmp", bufs=3)
        nc.sync.dma_start(tmp[:], ap3[:, c, :])
        nc.vector.tensor_copy(w16[:, c, :], tmp[:])
    return w16


def _build_decoder(ctx: ExitStack, tc_: tile.TileContext, L: int, io: dict):
    nc = tc_.nc
    x, out = io["x"], io["out"]

    const = ctx.enter_context(tc_.tile_pool(name="const", bufs=1))
    big = ctx.enter_context(tc_.tile_pool(name="big", bufs=1))

    ident16 = const.tile([P, P], F16)
    make_identity(nc, ident16[:])
    ident32 = const.tile([P, P], F32)
    make_identity(nc, ident32[:])

    # ---------------- persistent SBUF tensors ----------------
    x_nat = big.tile([P, BL, TC, I], F16)    # x[t%128, b, t//128, i]   64KB/par
    U_sb = big.tile([P, HC, T, BL], F16)     # U[h%128, h//128, t, b]   64KB/par

    # ---------------- state tiles (ping-pong via bufs=2 pools) ----------------
    state = ctx.enter_context(tc_.tile_pool(name="state", bufs=2))

    sT_f32 = state.tile([P, HC, BL], F32, tag="sT32", name="s0T_f32")
    sT_f16 = state.tile([P, HC, BL], F16, tag="sT16", name="s0T_f16")
    out_nat = state.tile([BL, O], F32, tag="out", name="out0_nat")
    outT_f16 = state.tile([P, BL], F16, tag="outT", name="out0T_f16")

    # ---------------- setup: weights, load x, transpose, U = x @ ua, s0 ------
    with tc_.tile_pool(name="setup", bufs=2) as stp, \
         tc_.tile_pool(name="setup1", bufs=1) as stp1, \
         tc_.tile_pool(name="wcast", bufs=1) as wcast, \
         tc_.tile_pool(name="stpsA", bufs=2, space="PSUM") as stpsA, \
         tc_.tile_pool(name="stpsB", bufs=3, space="PSUM") as stpsB, \
         tc_.tile_pool(name="stpsC", bufs=1, space="PSUM") as stpsC:

        # x projections needed for setup compute come FIRST so the x DMAs
        # and U matmuls are not stuck behind the step-weight DMA queue
        ua_sb = _load_weight_pkh(nc, stp1, "ua", io["ua"], IC, wcast)
        ws_sb = _load_weight_pkh(nc, stp1, "ws", io["ws"], IC, wcast)

        GB = 2  # batches per transpose group
        for g in range(BL // GB):
            xT_g = stp.tile([P, IC, GB, T], F16, tag="xTg", name="xT_g")
            for bi in range(GB):
                b = g * GB + bi
                for t_ in range(TC):
                    xdma = stp.tile([P, I], F32, tag="xdma", name="xdma",
                                    bufs=4)
                    nc.sync.dma_start(xdma[:], x[t_ * P:(t_ + 1) * P, b, :])
                    if t_ % 2 == 0:
                        nc.vector.tensor_copy(x_nat[:, b, t_, :], xdma[:])
                    else:
                        nc.scalar.copy(x_nat[:, b, t_, :], xdma[:])
                # transpose [t,i] tiles -> xT_g[i, t]
                for ic in range(IC):
                    tps = stpsA.tile([P, T], F16, tag="xtp", name="xtp")
                    for t_ in range(TC):
                        nc.tensor.transpose(
                            tps[:, t_ * P:(t_ + 1) * P],
                            x_nat[:, b, t_, ic * P:(ic + 1) * P], ident16[:])
                    if ic % 2 == 0:
                        nc.vector.tensor_copy(xT_g[:, ic, bi, :], tps[:])
                    else:
                        nc.scalar.copy(xT_g[:, ic, bi, :], tps[:])
                # U[:, hc, :, b] = sum_ic ua[ic]^T-chunk . xT
                for hc in range(HC):
                    ups = stpsB.tile([P, T], F32, tag="ups", name="ups")
                    for ic in range(IC):
                        nc.tensor.matmul(
                            ups[:], ua_sb[:, ic, hc * P:(hc + 1) * P],
                            xT_g[:, ic, bi, :],
                            start=(ic == 0), stop=(ic == IC - 1))
                    if hc % 2 == 0:
                        nc.vector.tensor_copy(U_sb[:, hc, :, b], ups[:])
                    else:
                        nc.scalar.copy(U_sb[:, hc, :, b], ups[:])

        # step weights, ordered by first use in the decode loop, with a
        # pipelined DMA/cast (bufs=3) so they finish before step 1 needs them
        def _load_w(pool, wname, ap, kc, scale=None):
            w16 = pool.tile([P, kc, H], F16, name=f"{wname}_sb")
            ap3 = ap.rearrange("(c p) h -> p c h", p=P)
            for c in range(kc):
                tmp = wcast.tile([P, H], F32, tag="wload",
                                 name=f"{wname}_f32tmp", bufs=3)
                nc.sync.dma_start(tmp[:], ap3[:, c, :])
                if scale is not None:
                    nc.vector.tensor_scalar_mul(w16[:, c, :], tmp[:], scale)
                elif c % 2 == 0:
                    nc.vector.tensor_copy(w16[:, c, :], tmp[:])
                else:
                    nc.scalar.copy(w16[:, c, :], tmp[:])
            return w16

        wa_sb = _load_w(const, "wa", io["wa"], HC)
        ur_sb = _load_w(const, "ur", io["ur"], HC)
        uz_sb = _load_w(const, "uz", io["uz"], HC)
        cr_sb = _load_w(const, "cr", io["cr"], IC)
        cz_sb = _load_w(const, "cz", io["cz"], IC)
        u0_sb = _load_w(const, "u0", io["u0"], HC, scale=0.5)
        c0_sb = _load_w(const, "c0", io["c0"], IC)

        # [O, H] gate input weights, zero-padded to K=128 partitions
        # (K<128 matmuls are unreliable: the PE contracts over the full
        # partition range, so unused partitions must be zero)
        wsmall = {}
        for nm in ("wr", "wz", "w0"):
            tmp = wcast.tile([O, H], F32, tag="wsload", name=f"{nm}_f32tmp", bufs=1)
            nc.sync.dma_start(tmp[:], io[nm])
            w16 = const.tile([P, H], F16, name=f"{nm}_sb")
            nc.vector.memset(w16[:], 0.0)
            nc.vector.tensor_copy(w16[:O, :], tmp[:])
            wsmall[nm] = w16

        # fc kept fp32 for output accuracy
        fcw_sb = const.tile([P, HC, O], F32)
        nc.sync.dma_start(fcw_sb[:],
                          io["fc_w"].rearrange("(c p) o -> p c o", p=P))
        fcb_sb = const.tile([BL, O], F32)
        nc.sync.dma_start(fcb_sb[:], io["fc_b"][None, :].to_broadcast((BL, O)))

        # va chunks [P, HC]: lhsT columns for the e-dot
        va_f32 = const.tile([P, HC], F32)
        nc.sync.dma_start(va_f32[:],
                          io["va"][:, 0].rearrange("(c p) -> p c", p=P))
        va_f16 = const.tile([P, HC], F16)
        nc.vector.tensor_copy(va_f16[:], va_f32[:])

        # ---- s0 = tanh(x0 @ ws) (transposed), out0 = s0 @ fc_w + fc_b ----
        x0_f32 = stp1.tile([BL, I], F32)
        nc.sync.dma_start(x0_f32[:], x[0, :, :])
        x0_f16 = stp1.tile([BL, I], F16)
        nc.vector.tensor_copy(x0_f16[:], x0_f32[:])
        x0T = stp1.tile([P, IC, BL], F16)
        x0ps = stpsA.tile([P, IC, BL], F16, tag="xtp", name="x0tp")
        for c in range(IC):
            nc.tensor.transpose(x0ps[:, c, :], x0_f16[:, c * P:(c + 1) * P],
                                ident16[:BL, :BL])
        nc.vector.tensor_copy(x0T[:], x0ps[:])

        s0T_ps = stpsC.tile([P, HC, BL], F32, name="s0T_ps")
        for hc in range(HC):
            for ic in range(IC):
                nc.tensor.matmul(
                    s0T_ps[:, hc, :], ws_sb[:, ic, hc * P:(hc + 1) * P],
                    x0T[:, ic, :], start=(ic == 0), stop=(ic == IC - 1))
        nc.scalar.activation(sT_f16[:], s0T_ps[:], AF.Tanh)
        nc.scalar.activation(sT_f32[:], s0T_ps[:], AF.Tanh)

    # ---------------- step-loop pools (opened after setup frees SBUF) -------
    work = ctx.enter_context(tc_.tile_pool(name="work", bufs=2))
    f16s = ctx.enter_context(tc_.tile_pool(name="f16s", bufs=2))
    vpool = ctx.enter_context(tc_.tile_pool(name="vpool", bufs=3))
    psE = ctx.enter_context(tc_.tile_pool(name="psE", bufs=1, space="PSUM"))
    psG = ctx.enter_context(tc_.tile_pool(name="psG", bufs=1, space="PSUM"))
    psW = ctx.enter_context(tc_.tile_pool(name="psW", bufs=1, space="PSUM"))
    psC = ctx.enter_context(tc_.tile_pool(name="psC", bufs=1, space="PSUM"))
    psS = ctx.enter_context(tc_.tile_pool(name="psS", bufs=1, space="PSUM"))

    # out0
    ops = psS.tile([BL, O], F32, tag="small", name="out0_ps")
    for kc in range(HC):
        nc.tensor.matmul(ops[:], sT_f32[:, kc, :], fcw_sb[:, kc, :],
                         start=(kc == 0), stop=(kc == HC - 1))
    nc.vector.tensor_tensor(out_nat[:], ops[:], fcb_sb[:], ALU.add)
    nc.sync.dma_start(out[0], out_nat[:])
    out_f16 = f16s.tile([BL, O], F16, tag="of16", name="out0_f16")
    nc.gpsimd.tensor_copy(out_f16[:], out_nat[:])
    otp = psS.tile([O, BL], F16, tag="small", name="out0T_ps")
    nc.tensor.transpose(otp[:], out_f16[:], ident16[:BL, :BL])
    nc.gpsimd.memset(outT_f16[:], 0.0)
    nc.vector.tensor_copy(outT_f16[:O, :], otp[:])

    # ---------------- decode steps ----------------
    outT_holder = [outT_f16]
    deferred = []
    for k in range(1, L):
        # --- sWaT[h, b] (transposed) + per-hc copies so V-add starts early --
        swps = psW.tile([P, HC, BL], F32, tag="swa", name="sw_ps")
        swaT = f16s.tile([P, HC, BL], F16, tag="swaT", name="swaT")
        for hc in range(HC):
            for kc in range(HC):
                nc.tensor.matmul(swps[:, hc, :],
                                 wa_sb[:, kc, hc * P:(hc + 1) * P],
                                 sT_f16[:, kc, :],
                                 start=(kc == 0), stop=(kc == HC - 1))
            if hc == 0:
                nc.vector.tensor_copy(swaT[:, 0, :], swps[:, 0, :])
        nc.vector.tensor_copy(swaT[:, 1:, :], swps[:, 1:, :])

        # --- early gate terms: deferred so the k-1 out-path (which feeds
        # outT) and these matmuls stay off the step-restart critical queues
        g_r = psG.tile([P, HC, BL], F32, tag="gr", name="r_ps")
        g_z = psG.tile([P, HC, BL], F32, tag="gz", name="z_ps")
        g_h = psG.tile([P, HC, BL], F32, tag="gh", name="h_ps")

        def _early_gates(g_r=g_r, g_z=g_z, g_h=g_h, sT_f16=sT_f16):
            oT = outT_holder[0]
            for ps, wo, uw in ((g_r, "wr", ur_sb), (g_z, "wz", uz_sb)):
                for hc in range(HC):
                    nc.tensor.matmul(ps[:, hc, :],
                                     wsmall[wo][:, hc * P:(hc + 1) * P],
                                     oT[:], start=(hc == 0), stop=False)
                    for kc in range(HC):
                        nc.tensor.matmul(ps[:, hc, :],
                                         uw[:, kc, hc * P:(hc + 1) * P],
                                         sT_f16[:, kc, :],
                                         start=False, stop=False)
            for hc in range(HC):
                nc.tensor.matmul(g_h[:, hc, :],
                                 wsmall["w0"][:, hc * P:(hc + 1) * P],
                                 oT[:], start=(hc == 0), stop=False)

        # --- attention: V = tanh(U + sWa) slabs; eT via lhsT=V (engine-free) --
        # th outer so the first half's e fixups hide under the second half's
        # tanh phase
        eT_ps = psC.tile([P, TC, BL], F32, tag="eT", name="eT_ps")
        e_nat = psE.tile([BL, T], F32, tag="e", name="e_nat")
        eT_sb = work.tile([P, TC, BL], F32, tag="eTsb", name="eT_sb")
        cT_ps = psC.tile([P, IC, BL], F32, tag="cT", name="cT_ps")
        aT_ps = psW.tile([P, TC, BL], F16, tag="swa", name="aT_ps")
        aT16 = f16s.tile([P, TC, BL], F16, tag="aT16", name="aT16")
        p16 = f16s.tile([BL, T], F16, tag="p", name="p16")
        negm = work.tile([BL, 1], F32, tag="negm", name="negm")
        TSUB = THL // P
        S_h = [None, None]
        # flash-style: half 0's softmax + context run under half 1's tanh;
        # p is exp(e - m0) (m0 = first-half max) and 1/S is folded into the
        # cT copy afterwards, so no full-width rescale sits in the tail
        S_parts = []
        S01 = work.tile([BL, 1], F32, tag="S01", name="S01")
        deferred.append(_early_gates)

        def _fixup(tcs, first, defer=False):
            # finalize e columns for t-chunks `tcs`: transpose to natural,
            # exp (bias = -m0), transpose a, and their context matmuls.
            # defer=True postpones the exp + downstream so the ACT queue
            # issues the NEXT tanh chunk first (in-order queue would stall)
            t0, t1 = tcs[0], tcs[-1] + 1
            nc.vector.tensor_copy(eT_sb[:, t0:t1, :], eT_ps[:, t0:t1, :])
            for tcc in tcs:
                nc.tensor.transpose(e_nat[:, tcc * P:(tcc + 1) * P],
                                    eT_sb[:, tcc, :], ident32[:])
            if first:
                nc.vector.tensor_reduce(negm[:], e_nat[:, t0 * P:t1 * P],
                                        axis=AX.X, op=ALU.max)
                nc.vector.tensor_scalar_mul(negm[:], negm[:], -1.0)

            def _finish():
                S_x = work.tile([BL, 1], F32, tag=f"S{len(S_parts)}",
                                name=f"S{len(S_parts)}")
                S_parts.append(S_x)
                nc.scalar.activation(p16[:, t0 * P:t1 * P],
                                     e_nat[:, t0 * P:t1 * P], AF.Exp,
                                     bias=negm[:], accum_out=S_x[:])
                for tcc in tcs:
                    nc.tensor.transpose(aT_ps[:, tcc, :],
                                        p16[:, tcc * P:(tcc + 1) * P],
                                        ident16[:BL, :BL])
                nc.vector.tensor_copy(aT16[:, t0:t1, :], aT_ps[:, t0:t1, :])
                for b in range(BL):
                    for ic in range(IC):
                        for tcc in tcs:
                            nc.tensor.matmul(
                                cT_ps[:, ic, b:b + 1],
                                x_nat[:, b, tcc, ic * P:(ic + 1) * P],
                                aT16[:, tcc, b:b + 1],
                                start=(first and b == 0 and ic == 0
                                       and tcc == tcs[0]),
                                stop=(tcc == TC - 1 and b == BL - 1
                                      and ic == IC - 1))
                if len(S_parts) == 2:
                    nc.vector.tensor_tensor(S01[:], S_parts[0][:],
                                            S_parts[1][:], ALU.add)
            if defer:
                deferred.append(_finish)
            else:
                _finish()

        for th in range(NTH):
            for hc in range(HC):
                v = vpool.tile([P, THL, BL], F16, tag="v", name="vslab")
                last = (th == NTH - 1 and hc == HC - 1)
                if th == 0 and hc == 0:
                    subs = ((0, 96), (96, THL))
                elif last:
                    subs = ((0, P), (P, THL))
                else:
                    subs = ((0, THL),)
                done_ts = 0
                for lo, hi in subs:
                    nc.vector.tensor_tensor(
                        v[:, lo:hi, :], U_sb[:, hc, th * THL + lo:
                                             th * THL + hi, :],
                        swaT[:, hc, None, :].to_broadcast((P, hi - lo, BL)),
                        ALU.add)
                    nc.scalar.activation(v[:, lo:hi, :], v[:, lo:hi, :],
                                         AF.Tanh)
                    for fin in deferred:
                        fin()
                    deferred.clear()
                    while (done_ts + 1) * P <= hi:
                        ts = done_ts
                        for b in range(BL):
                            nc.tensor.matmul(
                                eT_ps[:, th * TSUB + ts, b:b + 1],
                                v[:, ts * P:(ts + 1) * P, b],
                                va_f16[:, hc:hc + 1],
                                start=(th == 0 and hc == 0 and ts == 0
                                       and b == 0),
                                stop=(last and ts == TSUB - 1
                                      and b == BL - 1))
                        done_ts += 1
                        if last:
                            # finalize per t-chunk; ts=0 defers its exp
                            # until after the final tanh is issued
                            _fixup([th * TSUB + ts], first=False,
                                   defer=(ts == 0))
            if th == 0:
                _fixup([0, 1], first=True, defer=True)

        # --- 1/S broadcast across partitions, fold into the cT copy ---
        # (S01's transpose is hidden under tanh; only S1b's is post-exp)
        s01_row_ps = psS.tile([1, BL], F32, tag="small", name="s01_row_ps")
        nc.tensor.transpose(s01_row_ps[:], S01[:], ident32[:BL, :BL])
        s01_row = work.tile([1, BL], F32, tag="s01row", name="s01_row")
        nc.vector.tensor_copy(s01_row[:], s01_row_ps[:])
        s1b_row = psS.tile([1, BL], F32, tag="small", name="s1b_row_ps")
        nc.tensor.transpose(s1b_row[:], S_parts[2][:], ident32[:BL, :BL])
        s_row_sb = work.tile([1, BL], F32, tag="srow", name="s_row_sb")
        nc.vector.tensor_tensor(s_row_sb[:], s01_row[:], s1b_row[:], ALU.add)
        rs_row_sb = work.tile([1, BL], F32, tag="rsrow", name="rs_row_sb")
        nc.vector.reciprocal(rs_row_sb[:], s_row_sb[:])
        rSb = work.tile([P, BL], F32, tag="rSb", name="rSb")
        nc.gpsimd.partition_broadcast(rSb[:], rs_row_sb[:])
        cT16 = f16s.tile([P, IC, BL], F16, tag="cT16", name="cT16")
        nc.vector.tensor_tensor(
            cT16[:], cT_ps[:],
            rSb[:, None, :].to_broadcast((P, IC, BL)), ALU.mult)

        # --- late gate matmuls (c terms; then r*s term for h) ---
        for ps, cw in ((g_r, cr_sb), (g_z, cz_sb)):
            for hc in range(HC):
                for ic in range(IC):
                    nc.tensor.matmul(ps[:, hc, :],
                                     cw[:, ic, hc * P:(hc + 1) * P],
                                     cT16[:, ic, :],
                                     start=False,
                                     stop=(hc == HC - 1 and ic == IC - 1))
        for hc in range(HC):
            for ic in range(IC):
                nc.tensor.matmul(g_h[:, hc, :],
                                 c0_sb[:, ic, hc * P:(hc + 1) * P],
                                 cT16[:, ic, :], start=False, stop=False)
        th_r = work.tile([P, HC, BL], F32, tag="thr", name="th_r")
        nc.scalar.activation(th_r[:], g_r[:], AF.Tanh, scale=0.5)
        # rs' = (th_r + 1)*s  (u0 is pre-scaled by 0.5 so rs'@u0h == rs@u0)
        rsT16 = f16s.tile([P, HC, BL], F16, tag="rsT", name="rsT16")
        nc.vector.scalar_tensor_tensor(
            out=rsT16[:], in0=th_r[:], scalar=1.0, in1=sT_f32[:],
            op0=ALU.add, op1=ALU.mult)

        for hc in range(HC):
            for kc in range(HC):
                nc.tensor.matmul(g_h[:, hc, :],
                                 u0_sb[:, kc, hc * P:(hc + 1) * P],
                                 rsT16[:, kc, :], start=False,
                                 stop=(hc == HC - 1 and kc == HC - 1))
        th_z = work.tile([P, HC, BL], F32, tag="thz", name="th_z")
        nc.scalar.activation(th_z[:], g_z[:], AF.Tanh, scale=0.5)
        # s_new = s + (0.5*th_z+0.5)*(sh-s) = 0.5*a - c with
        # a = (th_z+1)*sh and c = 0.5*(th_z+1)*s - s; c only needs th_z so
        # it is computed while g_h finishes, leaving 2 DVE ops after tanh_sh
        b_pre = work.tile([P, HC, BL], F32, tag="sdiff", name="b_pre")
        nc.vector.scalar_tensor_tensor(
            out=b_pre[:], in0=th_z[:], scalar=1.0, in1=sT_f32[:],
            op0=ALU.add, op1=ALU.mult)
        c_pre = work.tile([P, HC, BL], F32, tag="zd", name="c_pre")
        nc.vector.scalar_tensor_tensor(
            out=c_pre[:], in0=b_pre[:], scalar=0.5, in1=sT_f32[:],
            op0=ALU.mult, op1=ALU.subtract)
        shT = work.tile([P, HC, BL], F32, tag="sh", name="shT")
        nc.scalar.activation(shT[:], g_h[:], AF.Tanh)
        a_post = work.tile([P, HC, BL], F32, tag="apost", name="a_post")
        nc.vector.scalar_tensor_tensor(
            out=a_post[:], in0=th_z[:], scalar=1.0, in1=shT[:],
            op0=ALU.add, op1=ALU.mult)
        s_new16 = state.tile([P, HC, BL], F16, tag="sT16", name=f"s{k}T_f16")
        nc.vector.scalar_tensor_tensor(
            out=s_new16[:], in0=a_post[:], scalar=0.5, in1=c_pre[:],
            op0=ALU.mult, op1=ALU.subtract)
        s_new32 = state.tile([P, HC, BL], F32, tag="sT32", name=f"s{k}T_f32")
        nc.vector.scalar_tensor_tensor(
            out=s_new32[:], in0=a_post[:], scalar=0.5, in1=c_pre[:],
            op0=ALU.mult, op1=ALU.subtract)
        if k == 1 and "dbg_swaT" in io:
            nc.sync.dma_start(io["dbg_swaT"], swaT[:])
            nc.sync.dma_start(io["dbg_eT"], eT_sb[:])
            nc.sync.dma_start(io["dbg_p"], pa16[:])
            nc.sync.dma_start(io["dbg_aT"], aT16[:])
            nc.sync.dma_start(io["dbg_cT"], cT16[:])
            nc.sync.dma_start(io["dbg_thr"], th_r[:])
            nc.sync.dma_start(io["dbg_sh"], shT[:])
            nc.sync.dma_start(io["dbg_s"], s_new32[:])
        sT_f32, sT_f16 = s_new32, s_new16

        # --- out = s @ fc_w + fc_b (deferred into the next step's tanh
        # phase so its DVE/PE ops stay off the step-restart chain) ---
        def _out_path(k=k, s32=sT_f32):
            ops = psS.tile([BL, O], F32, tag="small", name=f"out{k}_ps")
            for kc in range(HC):
                nc.tensor.matmul(ops[:], s32[:, kc, :], fcw_sb[:, kc, :],
                                 start=(kc == 0), stop=(kc == HC - 1))
            out_nat = state.tile([BL, O], F32, tag="out", name=f"out{k}_nat")
            nc.vector.tensor_tensor(out_nat[:], ops[:], fcb_sb[:], ALU.add)
            nc.sync.dma_start(out[k], out_nat[:])
            if k < L - 1:
                of16 = f16s.tile([BL, O], F16, tag="of16",
                                 name=f"out{k}_f16")
                nc.gpsimd.tensor_copy(of16[:], out_nat[:])
                otp = psS.tile([O, BL], F16, tag="small", name=f"out{k}T_ps")
                nc.tensor.transpose(otp[:], of16[:], ident16[:BL, :BL])
                oT = state.tile([P, BL], F16, tag="outT", name=f"out{k}T")
                nc.gpsimd.memset(oT[:], 0.0)
                nc.vector.tensor_copy(oT[:O, :], otp[:])
                outT_holder[0] = oT
        if k < L - 1:
            deferred.append(_out_path)
        else:
            _out_path()


_BUILT = {}


def _get_nc(L: int, debug: bool = False):
    key = (L, debug)
    if key in _BUILT:
        return _BUILT[key]
    nc = bacc.Bacc("TRN2", target_bir_lowering=False, debug=False,
                   enable_asserts=False, num_devices=NCORES)
    io = {}
    io["x"] = nc.dram_tensor("x", [T, BL, I], F32, kind="ExternalInput").ap()
    shapes = {"w0": [O, H], "wz": [O, H], "wr": [O, H], "ws": [I, H],
              "wa": [H, H], "ua": [I, H], "va": [H, 1], "u0": [H, H],
              "uz": [H, H], "ur": [H, H], "c0": [I, H], "cz": [I, H],
              "cr": [I, H], "fc_w": [H, O], "fc_b": [O]}
    for nm, shp in shapes.items():
        io[nm] = nc.dram_tensor(nm, shp, F32, kind="ExternalInput").ap()
    io["out"] = nc.dram_tensor("out", [L, BL, O], F32,
                               kind="ExternalOutput").ap()
    if debug:
        for nm, shp, dt in [("dbg_swaT", [P, HC, BL], F16),
                            ("dbg_eT", [P, TC, BL], F32),
                            ("dbg_p", [BL, T], F16),
                            ("dbg_aT", [P, TC, BL], F16),
                            ("dbg_cT", [P, IC, BL], F16),
                            ("dbg_thr", [P, HC, BL], F32),
                            ("dbg_sh", [P, HC, BL], F32),
                            ("dbg_s", [P, HC, BL], F32)]:
            io[nm] = nc.dram_tensor(nm, shp, dt, kind="ExternalOutput").ap()
    with tile.TileContext(nc) as tc_:
        with ExitStack() as ctx:
            _build_decoder(ctx, tc_, L, io)
    nc.compile()
    _BUILT[key] = (nc, io)
    return _BUILT[key]


def kernel(**inputs) -> np.ndarray:
    L = int(np.asarray(inputs["max_labels"]))
    nc, _ = _get_nc(L)
    x = np.ascontiguousarray(np.asarray(inputs["x"], dtype=np.float32))
    base = {nm: np.ascontiguousarray(np.asarray(inputs[nm], dtype=np.float32))
            for nm in WNAMES}
    base["fc_b"] = base["fc_b"].reshape(O)
    in_maps = []
    for c in range(NCORES):
        m = dict(base)
        m["x"] = np.ascontiguousarray(x[:, c * BL:(c + 1) * BL, :])
        in_maps.append(m)
    res = run_bass_kernel_spmd(nc, in_maps, core_ids=list(range(NCORES)))
    outs = [r["out"] for r in res.results]            # each [L, BL, O]
    full = np.concatenate([o.transpose(1, 0, 2) for o in outs], axis=0)
    return np.ascontiguousarray(full.astype(np.float32))


if __name__ == "__main__":
    import reference
    ins = reference.setup_inputs()
    got = kernel(**{k: np.asarray(v) if not isinstance(v, int) else v
                    for k, v in ins.items()})
    print("kernel output", got.shape, got.dtype)


# revision 33
# speedup vs baseline: 1.0037x; 1.0037x over previous
"""Trainium2 Bass kernel for a Bahdanau-attention GRU decoder.

Reference computation (T=512, B=128, I=H=512, O=12, L=max_labels=16):
    s0 = tanh(x[0] @ ws);  out0 = s0 @ fc_w + fc_b
    U  = einsum('tbi,ih->tbh', x, ua)            # precomputed once
    per step:
        e  = einsum('tbh,h->tb', tanh(s @ wa + U), va)
        a  = softmax(e, axis=t)
        c  = einsum('tb,tbi->bi', a, x)
        r  = sigmoid(out @ wr + s @ ur + c @ cr)
        z  = sigmoid(out @ wz + s @ uz + c @ cz)
        sh = tanh(out @ w0 + (r*s) @ u0 + c @ c0)
        s  = (1-z)*s + z*sh;  out = s @ fc_w + fc_b
    returns [B, L, O]

Sharding: data-parallel over batch B across 8 cores (BL=16 per core), all
weights replicated; no collectives.  Per core, x (fp16, [t%128, b, t//128, i])
and U (fp16, [h%128, h//128, t, b]) are SBUF-resident so the recurrence never
touches HBM.

Step-loop engine split (ACT-bound; tanh of [T,BL,H] is the floor):
  DVE : V = U + broadcast(sWa) slabs (fp16 2x), softmax scalars, gate combines
  ACT : tanh(V) on [128, 256*16] slabs; exp; gate tanh (sigmoid via tanh)
  Pool: small PSUM->SBUF copies + gate adds (keeps DVE under the ACT floor)
  PE  : everything in transposed [h, b] space so matmuls are cheap:
        e-dot via lhsT=va chunk (m=1, N=256); context via lhsT=x chunks
        (Ldweights are engine-free, N=1); gate matmuls lhsT=weight chunks
        rhs=state columns (N=16); s_newT produced directly (no transposes).
"""

import numpy as np
from contextlib import ExitStack

import concourse.bass as bass
import concourse.mybir as mybir
import concourse.tile as tile
from concourse import bacc
from concourse.bass_utils import run_bass_kernel_spmd
from concourse.masks import make_identity

F32 = mybir.dt.float32
F16 = mybir.dt.float16
AF = mybir.ActivationFunctionType
ALU = mybir.AluOpType
AX = mybir.AxisListType

T, B, I, H, O = 512, 128, 512, 512, 12
P = 128
NCORES = 8
BL = B // NCORES        # 16 batches per core
HC = H // P             # 4 h-chunks
IC = I // P             # 4 i-chunks
TC = T // P             # 4 t-chunks
NTH = 2                 # t-halves for the attention slabs
THL = T // NTH          # 256

WNAMES = ["w0", "wz", "wr", "ws", "wa", "ua", "va", "u0", "uz", "ur",
          "c0", "cz", "cr", "fc_w", "fc_b"]


def _load_weight_pkh(nc, pool, wname, ap, kc, cast_pool, dtype=F16):
    """DRAM [K, H] fp32 -> SBUF [P, kc, H] in `dtype` (cast via DVE copy)."""
    w16 = pool.tile([P, kc, H], dtype, name=f"{wname}_sb")
    ap3 = ap.rearrange("(c p) h -> p c h", p=P)
    for c in range(kc):
        tmp = cast_pool.tile([P, H], F32, tag="wload",
                             name=f"{wname}_f32tmp", bufs=3)
        nc.sync.dma_start(tmp[:], ap3[:, c, :])
        nc.vector.tensor_copy(w16[:, c, :], tmp[:])
    return w16


def _build_decoder(ctx: ExitStack, tc_: tile.TileContext, L: int, io: dict):
    nc = tc_.nc
    x, out = io["x"], io["out"]

    const = ctx.enter_context(tc_.tile_pool(name="const", bufs=1))
    big = ctx.enter_context(tc_.tile_pool(name="big", bufs=1))

    ident16 = const.tile([P, P], F16)
    make_identity(nc, ident16[:])
    ident32 = const.tile([P, P], F32)
    make_identity(nc, ident32[:])

    # ---------------- persistent SBUF tensors ----------------
    x_nat = big.tile([P, BL, TC, I], F16)    # x[t%128, b, t//128, i]   64KB/par
    U_sb = big.tile([P, HC, T, BL], F16)     # U[h%128, h//128, t, b]   64KB/par

    # ---------------- state tiles (ping-pong via bufs=2 pools) ----------------
    state = ctx.enter_context(tc_.tile_pool(name="state", bufs=2))

    sT_f32 = state.tile([P, HC, BL], F32, tag="sT32", name="s0T_f32")
    sT_f16 = state.tile([P, HC, BL], F16, tag="sT16", name="s0T_f16")
    out_nat = state.tile([BL, O], F32, tag="out", name="out0_nat")
    outT_f16 = state.tile([P, BL], F16, tag="outT", name="out0T_f16")

    # ---------------- setup: weights, load x, transpose, U = x @ ua, s0 ------
    with tc_.tile_pool(name="setup", bufs=2) as stp, \
         tc_.tile_pool(name="setup1", bufs=1) as stp1, \
         tc_.tile_pool(name="wcast", bufs=1) as wcast, \
         tc_.tile_pool(name="stpsA", bufs=3, space="PSUM") as stpsA, \
         tc_.tile_pool(name="stpsB", bufs=3, space="PSUM") as stpsB, \
         tc_.tile_pool(name="stpsC", bufs=1, space="PSUM") as stpsC:

        # x projections needed for setup compute come FIRST so the x DMAs
        # and U matmuls are not stuck behind the step-weight DMA queue
        ua_sb = _load_weight_pkh(nc, stp1, "ua", io["ua"], IC, wcast)
        ws_sb = _load_weight_pkh(nc, stp1, "ws", io["ws"], IC, wcast)

        GB = 2  # batches per transpose group
        for g in range(BL // GB):
            xT_g = stp.tile([P, IC, GB, T], F16, tag="xTg", name="xT_g")
            for bi in range(GB):
                b = g * GB + bi
                for t_ in range(TC):
                    xdma = stp.tile([P, I], F32, tag="xdma", name="xdma",
                                    bufs=4)
                    nc.sync.dma_start(xdma[:], x[t_ * P:(t_ + 1) * P, b, :])
                    if t_ % 2 == 0:
                        nc.vector.tensor_copy(x_nat[:, b, t_, :], xdma[:])
                    else:
                        nc.scalar.copy(x_nat[:, b, t_, :], xdma[:])
                # transpose [t,i] tiles -> xT_g[i, t]
                for ic in range(IC):
                    tps = stpsA.tile([P, T], F16, tag="xtp", name="xtp")
                    for t_ in range(TC):
                        nc.tensor.transpose(
                            tps[:, t_ * P:(t_ + 1) * P],
                            x_nat[:, b, t_, ic * P:(ic + 1) * P], ident16[:])
                    if ic % 2 == 0:
                        nc.vector.tensor_copy(xT_g[:, ic, bi, :], tps[:])
                    else:
                        nc.scalar.copy(xT_g[:, ic, bi, :], tps[:])
                # U[:, hc, :, b] = sum_ic ua[ic]^T-chunk . xT
                for hc in range(HC):
                    ups = stpsB.tile([P, T], F32, tag="ups", name="ups")
                    for ic in range(IC):
                        nc.tensor.matmul(
                            ups[:], ua_sb[:, ic, hc * P:(hc + 1) * P],
                            xT_g[:, ic, bi, :],
                            start=(ic == 0), stop=(ic == IC - 1))
                    if hc % 2 == 0:
                        nc.vector.tensor_copy(U_sb[:, hc, :, b], ups[:])
                    else:
                        nc.scalar.copy(U_sb[:, hc, :, b], ups[:])

        # step weights, ordered by first use in the decode loop, with a
        # pipelined DMA/cast (bufs=3) so they finish before step 1 needs them
        def _load_w(pool, wname, ap, kc, scale=None):
            w16 = pool.tile([P, kc, H], F16, name=f"{wname}_sb")
            ap3 = ap.rearrange("(c p) h -> p c h", p=P)
            for c in range(kc):
                tmp = wcast.tile([P, H], F32, tag="wload",
                                 name=f"{wname}_f32tmp", bufs=3)
                nc.sync.dma_start(tmp[:], ap3[:, c, :])
                if scale is not None:
                    nc.vector.tensor_scalar_mul(w16[:, c, :], tmp[:], scale)
                elif c % 2 == 0:
                    nc.vector.tensor_copy(w16[:, c, :], tmp[:])
                else:
                    nc.scalar.copy(w16[:, c, :], tmp[:])
            return w16

        wa_sb = _load_w(const, "wa", io["wa"], HC)
        ur_sb = _load_w(const, "ur", io["ur"], HC)
        uz_sb = _load_w(const, "uz", io["uz"], HC)
        cr_sb = _load_w(const, "cr", io["cr"], IC)
        cz_sb = _load_w(const, "cz", io["cz"], IC)
        u0_sb = _load_w(const, "u0", io["u0"], HC, scale=0.5)
        c0_sb = _load_w(const, "c0", io["c0"], IC)

        # [O, H] gate input weights, zero-padded to K=128 partitions
        # (K<128 matmuls are unreliable: the PE contracts over the full
        # partition range, so unused partitions must be zero)
        wsmall = {}
        for nm in ("wr", "wz", "w0"):
            tmp = wcast.tile([O, H], F32, tag="wsload", name=f"{nm}_f32tmp", bufs=1)
            nc.sync.dma_start(tmp[:], io[nm])
            w16 = const.tile([P, H], F16, name=f"{nm}_sb")
            nc.vector.memset(w16[:], 0.0)
            nc.vector.tensor_copy(w16[:O, :], tmp[:])
            wsmall[nm] = w16

        # fc kept fp32 for output accuracy
        fcw_sb = const.tile([P, HC, O], F32)
        nc.sync.dma_start(fcw_sb[:],
                          io["fc_w"].rearrange("(c p) o -> p c o", p=P))
        fcb_sb = const.tile([BL, O], F32)
        nc.sync.dma_start(fcb_sb[:], io["fc_b"][None, :].to_broadcast((BL, O)))

        # va chunks [P, HC]: lhsT columns for the e-dot
        va_f32 = const.tile([P, HC], F32)
        nc.sync.dma_start(va_f32[:],
                          io["va"][:, 0].rearrange("(c p) -> p c", p=P))
        va_f16 = const.tile([P, HC], F16)
        nc.vector.tensor_copy(va_f16[:], va_f32[:])

        # ---- s0 = tanh(x0 @ ws) (transposed), out0 = s0 @ fc_w + fc_b ----
        x0_f32 = stp1.tile([BL, I], F32)
        nc.sync.dma_start(x0_f32[:], x[0, :, :])
        x0_f16 = stp1.tile([BL, I], F16)
        nc.vector.tensor_copy(x0_f16[:], x0_f32[:])
        x0T = stp1.tile([P, IC, BL], F16)
        x0ps = stpsA.tile([P, IC, BL], F16, tag="xtp", name="x0tp")
        for c in range(IC):
            nc.tensor.transpose(x0ps[:, c, :], x0_f16[:, c * P:(c + 1) * P],
                                ident16[:BL, :BL])
        nc.vector.tensor_copy(x0T[:], x0ps[:])

        s0T_ps = stpsC.tile([P, HC, BL], F32, name="s0T_ps")
        for hc in range(HC):
            for ic in range(IC):
                nc.tensor.matmul(
                    s0T_ps[:, hc, :], ws_sb[:, ic, hc * P:(hc + 1) * P],
                    x0T[:, ic, :], start=(ic == 0), stop=(ic == IC - 1))
        nc.scalar.activation(sT_f16[:], s0T_ps[:], AF.Tanh)
        nc.scalar.activation(sT_f32[:], s0T_ps[:], AF.Tanh)

    # ---------------- step-loop pools (opened after setup frees SBUF) -------
    work = ctx.enter_context(tc_.tile_pool(name="work", bufs=2))
    f16s = ctx.enter_context(tc_.tile_pool(name="f16s", bufs=2))
    vpool = ctx.enter_context(tc_.tile_pool(name="vpool", bufs=3))
    psE = ctx.enter_context(tc_.tile_pool(name="psE", bufs=1, space="PSUM"))
    psG = ctx.enter_context(tc_.tile_pool(name="psG", bufs=1, space="PSUM"))
    psW = ctx.enter_context(tc_.tile_pool(name="psW", bufs=1, space="PSUM"))
    psC = ctx.enter_context(tc_.tile_pool(name="psC", bufs=1, space="PSUM"))
    psS = ctx.enter_context(tc_.tile_pool(name="psS", bufs=1, space="PSUM"))

    # out0
    ops = psS.tile([BL, O], F32, tag="small", name="out0_ps")
    for kc in range(HC):
        nc.tensor.matmul(ops[:], sT_f32[:, kc, :], fcw_sb[:, kc, :],
                         start=(kc == 0), stop=(kc == HC - 1))
    nc.vector.tensor_tensor(out_nat[:], ops[:], fcb_sb[:], ALU.add)
    nc.sync.dma_start(out[0], out_nat[:])
    out_f16 = f16s.tile([BL, O], F16, tag="of16", name="out0_f16")
    nc.gpsimd.tensor_copy(out_f16[:], out_nat[:])
    otp = psS.tile([O, BL], F16, tag="small", name="out0T_ps")
    nc.tensor.transpose(otp[:], out_f16[:], ident16[:BL, :BL])
    nc.gpsimd.memset(outT_f16[:], 0.0)
    nc.vector.tensor_copy(outT_f16[:O, :], otp[:])

    # ---------------- decode steps ----------------
    outT_holder = [outT_f16]
    deferred = []
    for k in range(1, L):
        # --- sWaT[h, b] (transposed) + per-hc copies so V-add starts early --
        swps = psW.tile([P, HC, BL], F32, tag="swa", name="sw_ps")
        swaT = f16s.tile([P, HC, BL], F16, tag="swaT", name="swaT")
        for hc in range(HC):
            for kc in range(HC):
                nc.tensor.matmul(swps[:, hc, :],
                                 wa_sb[:, kc, hc * P:(hc + 1) * P],
                                 sT_f16[:, kc, :],
                                 start=(kc == 0), stop=(kc == HC - 1))
            if hc == 0:
                nc.vector.tensor_copy(swaT[:, 0, :], swps[:, 0, :])
        nc.vector.tensor_copy(swaT[:, 1:, :], swps[:, 1:, :])

        # --- early gate terms: deferred so the k-1 out-path (which feeds
        # outT) and these matmuls stay off the step-restart critical queues
        g_r = psG.tile([P, HC, BL], F32, tag="gr", name="r_ps")
        g_z = psG.tile([P, HC, BL], F32, tag="gz", name="z_ps")
        g_h = psG.tile([P, HC, BL], F32, tag="gh", name="h_ps")

        def _early_gates(g_r=g_r, g_z=g_z, g_h=g_h, sT_f16=sT_f16):
            oT = outT_holder[0]
            for ps, wo, uw in ((g_r, "wr", ur_sb), (g_z, "wz", uz_sb)):
                for hc in range(HC):
                    nc.tensor.matmul(ps[:, hc, :],
                                     wsmall[wo][:, hc * P:(hc + 1) * P],
                                     oT[:], start=(hc == 0), stop=False)
                    for kc in range(HC):
                        nc.tensor.matmul(ps[:, hc, :],
                                         uw[:, kc, hc * P:(hc + 1) * P],
                                         sT_f16[:, kc, :],
                                         start=False, stop=False)
            for hc in range(HC):
                nc.tensor.matmul(g_h[:, hc, :],
                                 wsmall["w0"][:, hc * P:(hc + 1) * P],
                                 oT[:], start=(hc == 0), stop=False)

        # --- attention: V = tanh(U + sWa) slabs; eT via lhsT=V (engine-free) --
        # th outer so the first half's e fixups hide under the second half's
        # tanh phase
        eT_ps = psC.tile([P, TC, BL], F32, tag="eT", name="eT_ps")
        e_nat = psE.tile([BL, T], F32, tag="e", name="e_nat")
        eT_sb = work.tile([P, TC, BL], F32, tag="eTsb", name="eT_sb")
        cT_ps = psC.tile([P, IC, BL], F32, tag="cT", name="cT_ps")
        aT_ps = psW.tile([P, TC, BL], F16, tag="swa", name="aT_ps")
        aT16 = f16s.tile([P, TC, BL], F16, tag="aT16", name="aT16")
        p16 = f16s.tile([BL, T], F16, tag="p", name="p16")
        negm = work.tile([BL, 1], F32, tag="negm", name="negm")
        TSUB = THL // P
        S_h = [None, None]
        # flash-style: half 0's softmax + context run under half 1's tanh;
        # p is exp(e - m0) (m0 = first-half max) and 1/S is folded into the
        # cT copy afterwards, so no full-width rescale sits in the tail
        S_parts = []
        S01 = work.tile([BL, 1], F32, tag="S01", name="S01")
        deferred.append(_early_gates)

        def _fixup(tcs, first, defer=False):
            # finalize e columns for t-chunks `tcs`: transpose to natural,
            # exp (bias = -m0), transpose a, and their context matmuls.
            # defer=True postpones the exp + downstream so the ACT queue
            # issues the NEXT tanh chunk first (in-order queue would stall)
            t0, t1 = tcs[0], tcs[-1] + 1
            nc.vector.tensor_copy(eT_sb[:, t0:t1, :], eT_ps[:, t0:t1, :])
            for tcc in tcs:
                nc.tensor.transpose(e_nat[:, tcc * P:(tcc + 1) * P],
                                    eT_sb[:, tcc, :], ident32[:])
            if first:
                nc.vector.tensor_reduce(negm[:], e_nat[:, t0 * P:t1 * P],
                                        axis=AX.X, op=ALU.max)
                nc.vector.tensor_scalar_mul(negm[:], negm[:], -1.0)

            def _finish():
                S_x = work.tile([BL, 1], F32, tag=f"S{len(S_parts)}",
                                name=f"S{len(S_parts)}")
                S_parts.append(S_x)
                nc.scalar.activation(p16[:, t0 * P:t1 * P],
                                     e_nat[:, t0 * P:t1 * P], AF.Exp,
                                     bias=negm[:], accum_out=S_x[:])
                for tcc in tcs:
                    nc.tensor.transpose(aT_ps[:, tcc, :],
                                        p16[:, tcc * P:(tcc + 1) * P],
                                        ident16[:BL, :BL])
                nc.vector.tensor_copy(aT16[:, t0:t1, :], aT_ps[:, t0:t1, :])
                for b in range(BL):
                    for ic in range(IC):
                        for tcc in tcs:
                            nc.tensor.matmul(
                                cT_ps[:, ic, b:b + 1],
                                x_nat[:, b, tcc, ic * P:(ic + 1) * P],
                                aT16[:, tcc, b:b + 1],
                                start=(first and b == 0 and ic == 0
                                       and tcc == tcs[0]),
                                stop=(tcc == TC - 1 and b == BL - 1
                                      and ic == IC - 1))
                if len(S_parts) == 2:
                    nc.vector.tensor_tensor(S01[:], S_parts[0][:],
                                            S_parts[1][:], ALU.add)
            if defer:
                deferred.append(_finish)
            else:
                _finish()

        for th in range(NTH):
            for hc in range(HC):
                v = vpool.tile([P, THL, BL], F16, tag="v", name="vslab")
                last = (th == NTH - 1 and hc == HC - 1)
                if th == 0 and hc == 0:
                    subs = ((0, 96), (96, THL))
                elif last:
                    subs = ((0, P), (P, THL))
                else:
                    subs = ((0, THL),)
                done_ts = 0
                for lo, hi in subs:
                    nc.vector.tensor_tensor(
                        v[:, lo:hi, :], U_sb[:, hc, th * THL + lo:
                                             th * THL + hi, :],
                        swaT[:, hc, None, :].to_broadcast((P, hi - lo, BL)),
                        ALU.add)
                    nc.scalar.activation(v[:, lo:hi, :], v[:, lo:hi, :],
                                         AF.Tanh)
                    for fin in deferred:
                        fin()
                    deferred.clear()
                    while (done_ts + 1) * P <= hi:
                        ts = done_ts
                        for b in range(BL):
                            nc.tensor.matmul(
                                eT_ps[:, th * TSUB + ts, b:b + 1],
                                v[:, ts * P:(ts + 1) * P, b],
                                va_f16[:, hc:hc + 1],
                                start=(th == 0 and hc == 0 and ts == 0
                                       and b == 0),
                                stop=(last and ts == TSUB - 1
                                      and b == BL - 1))
                        done_ts += 1
                        if last:
                            # finalize per t-chunk; ts=0 defers its exp
                            # until after the final tanh is issued
                            _fixup([th * TSUB + ts], first=False,
                                   defer=(ts == 0))
            if th == 0:
                _fixup([0, 1], first=True, defer=True)

        # --- 1/S broadcast across partitions, fold into the cT copy ---
        # (S01's transpose is hidden under tanh; only S1b's is post-exp)
        s01_row_ps = psS.tile([1, BL], F32, tag="small", name="s01_row_ps")
        nc.tensor.transpose(s01_row_ps[:], S01[:], ident32[:BL, :BL])
        s01_row = work.tile([1, BL], F32, tag="s01row", name="s01_row")
        nc.vector.tensor_copy(s01_row[:], s01_row_ps[:])
        s1b_row = psS.tile([1, BL], F32, tag="small", name="s1b_row_ps")
        nc.tensor.transpose(s1b_row[:], S_parts[2][:], ident32[:BL, :BL])
        s_row_sb = work.tile([1, BL], F32, tag="srow", name="s_row_sb")
        nc.vector.tensor_tensor(s_row_sb[:], s01_row[:], s1b_row[:], ALU.add)
        rs_row_sb = work.tile([1, BL], F32, tag="rsrow", name="rs_row_sb")
        nc.vector.reciprocal(rs_row_sb[:], s_row_sb[:])
        rSb = work.tile([P, BL], F32, tag="rSb", name="rSb")
        nc.gpsimd.partition_broadcast(rSb[:], rs_row_sb[:])
        cT16 = f16s.tile([P, IC, BL], F16, tag="cT16", name="cT16")
        nc.vector.tensor_tensor(
            cT16[:], cT_ps[:],
            rSb[:, None, :].to_broadcast((P, IC, BL)), ALU.mult)

        # --- late gate matmuls (c terms; then r*s term for h) ---
        for ps, cw in ((g_r, cr_sb), (g_z, cz_sb)):
            for hc in range(HC):
                for ic in range(IC):
                    nc.tensor.matmul(ps[:, hc, :],
                                     cw[:, ic, hc * P:(hc + 1) * P],
                                     cT16[:, ic, :],
                                     start=False,
                                     stop=(hc == HC - 1 and ic == IC - 1))
        for hc in range(HC):
            for ic in range(IC):
                nc.tensor.matmul(g_h[:, hc, :],
                                 c0_sb[:, ic, hc * P:(hc + 1) * P],
                                 cT16[:, ic, :], start=False, stop=False)
        th_r = work.tile([P, HC, BL], F32, tag="thr", name="th_r")
        nc.scalar.activation(th_r[:], g_r[:], AF.Tanh, scale=0.5)
        # rs' = (th_r + 1)*s  (u0 is pre-scaled by 0.5 so rs'@u0h == rs@u0)
        rsT16 = f16s.tile([P, HC, BL], F16, tag="rsT", name="rsT16")
        nc.vector.scalar_tensor_tensor(
            out=rsT16[:], in0=th_r[:], scalar=1.0, in1=sT_f32[:],
            op0=ALU.add, op1=ALU.mult)

        for hc in range(HC):
            for kc in range(HC):
                nc.tensor.matmul(g_h[:, hc, :],
                                 u0_sb[:, kc, hc * P:(hc + 1) * P],
                                 rsT16[:, kc, :], start=False,
                                 stop=(hc == HC - 1 and kc == HC - 1))
        th_z = work.tile([P, HC, BL], F32, tag="thz", name="th_z")
        nc.scalar.activation(th_z[:], g_z[:], AF.Tanh, scale=0.5)
        # s_new = s + (0.5*th_z+0.5)*(sh-s) = 0.5*a - c with
        # a = (th_z+1)*sh and c = 0.5*(th_z+1)*s - s; c only needs th_z so
        # it is computed while g_h finishes, leaving 2 DVE ops after tanh_sh
        b_pre = work.tile([P, HC, BL], F32, tag="sdiff", name="b_pre")
        nc.vector.scalar_tensor_tensor(
            out=b_pre[:], in0=th_z[:], scalar=1.0, in1=sT_f32[:],
            op0=ALU.add, op1=ALU.mult)
        c_pre = work.tile([P, HC, BL], F32, tag="zd", name="c_pre")
        nc.vector.scalar_tensor_tensor(
            out=c_pre[:], in0=b_pre[:], scalar=0.5, in1=sT_f32[:],
            op0=ALU.mult, op1=ALU.subtract)
        shT = work.tile([P, HC, BL], F32, tag="sh", name="shT")
        nc.scalar.activation(shT[:], g_h[:], AF.Tanh)
        a_post = work.tile([P, HC, BL], F32, tag="apost", name="a_post")
        nc.vector.scalar_tensor_tensor(
            out=a_post[:], in0=th_z[:], scalar=1.0, in1=shT[:],
            op0=ALU.add, op1=ALU.mult)
        s_new16 = state.tile([P, HC, BL], F16, tag="sT16", name=f"s{k}T_f16")
        nc.vector.scalar_tensor_tensor(
            out=s_new16[:], in0=a_post[:], scalar=0.5, in1=c_pre[:],
            op0=ALU.mult, op1=ALU.subtract)
        s_new32 = state.tile([P, HC, BL], F32, tag="sT32", name=f"s{k}T_f32")
        nc.vector.scalar_tensor_tensor(
            out=s_new32[:], in0=a_post[:], scalar=0.5, in1=c_pre[:],
            op0=ALU.mult, op1=ALU.subtract)
        if k == 1 and "dbg_swaT" in io:
            nc.sync.dma_start(io["dbg_swaT"], swaT[:])
            nc.sync.dma_start(io["dbg_eT"], eT_sb[:])
            nc.sync.dma_start(io["dbg_p"], pa16[:])
            nc.sync.dma_start(io["dbg_aT"], aT16[:])
            nc.sync.dma_start(io["dbg_cT"], cT16[:])
            nc.sync.dma_start(io["dbg_thr"], th_r[:])
            nc.sync.dma_start(io["dbg_sh"], shT[:])
            nc.sync.dma_start(io["dbg_s"], s_new32[:])
        sT_f32, sT_f16 = s_new32, s_new16

        # --- out = s @ fc_w + fc_b (deferred into the next step's tanh
        # phase so its DVE/PE ops stay off the step-restart chain) ---
        def _out_path(k=k, s32=sT_f32):
            ops = psS.tile([BL, O], F32, tag="small", name=f"out{k}_ps")
            for kc in range(HC):
                nc.tensor.matmul(ops[:], s32[:, kc, :], fcw_sb[:, kc, :],
                                 start=(kc == 0), stop=(kc == HC - 1))
            out_nat = state.tile([BL, O], F32, tag="out", name=f"out{k}_nat")
            nc.vector.tensor_tensor(out_nat[:], ops[:], fcb_sb[:], ALU.add)
            nc.sync.dma_start(out[k], out_nat[:])
            if k < L - 1:
                of16 = f16s.tile([BL, O], F16, tag="of16",
                                 name=f"out{k}_f16")
                nc.gpsimd.tensor_copy(of16[:], out_nat[:])
                otp = psS.tile([O, BL], F16, tag="small", name=f"out{k}T_ps")
                nc.tensor.transpose(otp[:], of16[:], ident16[:BL, :BL])
                oT = state.tile([P, BL], F16, tag="outT", name=f"out{k}T")
                nc.gpsimd.memset(oT[:], 0.0)
                nc.vector.tensor_copy(oT[:O, :], otp[:])
                outT_holder[0] = oT
        if k < L - 1:
            deferred.append(_out_path)
        else:
            _out_path()


_BUILT = {}


def _get_nc(L: int, debug: bool = False):
    key = (L, debug)
    if key in _BUILT:
        return _BUILT[key]
    nc = bacc.Bacc("TRN2", target_bir_lowering=False, debug=False,
                   enable_asserts=False, num_devices=NCORES)
    io = {}
    io["x"] = nc.dram_tensor("x", [T, BL, I], F32, kind="ExternalInput").ap()
    shapes = {"w0": [O, H], "wz": [O, H], "wr": [O, H], "ws": [I, H],
              "wa": [H, H], "ua": [I, H], "va": [H, 1], "u0": [H, H],
              "uz": [H, H], "ur": [H, H], "c0": [I, H], "cz": [I, H],
              "cr": [I, H], "fc_w": [H, O], "fc_b": [O]}
    for nm, shp in shapes.items():
        io[nm] = nc.dram_tensor(nm, shp, F32, kind="ExternalInput").ap()
    io["out"] = nc.dram_tensor("out", [L, BL, O], F32,
                               kind="ExternalOutput").ap()
    if debug:
        for nm, shp, dt in [("dbg_swaT", [P, HC, BL], F16),
                            ("dbg_eT", [P, TC, BL], F32),
                            ("dbg_p", [BL, T], F16),
                            ("dbg_aT", [P, TC, BL], F16),
                            ("dbg_cT", [P, IC, BL], F16),
                            ("dbg_thr", [P, HC, BL], F32),
                            ("dbg_sh", [P, HC, BL], F32),
                            ("dbg_s", [P, HC, BL], F32)]:
            io[nm] = nc.dram_tensor(nm, shp, dt, kind="ExternalOutput").ap()
    with tile.TileContext(nc) as tc_:
        with ExitStack() as ctx:
            _build_decoder(ctx, tc_, L, io)
    nc.compile()
    _BUILT[key] = (nc, io)
    return _BUILT[key]


def kernel(**inputs) -> np.ndarray:
    L = int(np.asarray(inputs["max_labels"]))
    nc, _ = _get_nc(L)
    x = np.ascontiguousarray(np.asarray(inputs["x"], dtype=np.float32))
    base = {nm: np.ascontiguousarray(np.asarray(inputs[nm], dtype=np.float32))
            for nm in WNAMES}
    base["fc_b"] = base["fc_b"].reshape(O)
    in_maps = []
    for c in range(NCORES):
        m = dict(base)
        m["x"] = np.ascontiguousarray(x[:, c * BL:(c + 1) * BL, :])
        in_maps.append(m)
    res = run_bass_kernel_spmd(nc, in_maps, core_ids=list(range(NCORES)))
    outs = [r["out"] for r in res.results]            # each [L, BL, O]
    full = np.concatenate([o.transpose(1, 0, 2) for o in outs], axis=0)
    return np.ascontiguousarray(full.astype(np.float32))


if __name__ == "__main__":
    import reference
    ins = reference.setup_inputs()
    got = kernel(**{k: np.asarray(v) if not isinstance(v, int) else v
                    for k, v in ins.items()})
    print("kernel output", got.shape, got.dtype)
